# revision 2
# baseline (speedup 1.0000x reference)
"""AlphaCompositor Trainium2 kernel (8-core SPMD, data-parallel over batch N).

Reference computation:
    valid   = fragments >= 0
    a       = where(valid, alphas, 0)
    weights = a * exclusive_cumprod(1 - a, axis=K)
    out[n,c,h,w] = sum_k weights[n,k,h,w] * features[c, fragments[n,k,h,w]]

Device strategy (per core = one image n):
  - Gather via SWDGE dma_gather (Ant extended instr), 1024 indices per
    instruction (HW ring limit is between 1024 and 2048), 512 instructions
    per core. This amortizes the ~1us/instruction SWDGE fixed cost that
    limited the 1-offset-per-partition indirect-DMA baseline to 5.8 ms.
  - dma_gather indices are int16 (max 32767) but V=100000 rows, so the host
    packs 4 adjacent feature rows per 256B bf16 table element
    (featP[j] = rows 4j..4j+3); element index = frag>>2 < 25000. The 4-way
    select (by frag&3) is folded into the weight multiply: host ships
    wm_r = w * (frag&3 == r) and DVE computes
    sum_r g[..., r*32:(r+1)*32] * wm_r, then a 3-level tree add over K.
  - Weights host-computed, pre-masked, pair-duplicated (innermost dim 2) so
    the DVE multiply keeps the 2x bf16 packed mode (innermost step 1).
  - Raw engine blocks with manual semaphores: Pool runs gathers, SP (HWDGE)
    streams wm chunks + writes outputs, DVE does the selects/reduction.
    Tile is not used because it does not manage dma_gather's DMA-completion
    semaphore.
"""

import os
import sys

for _p in ("/opt/trn_rl_repo",):
    if os.path.isdir(_p) and _p not in sys.path:
        sys.path.insert(0, _p)

from contextlib import ExitStack

import ml_dtypes
import numpy as np

from concourse import bass, bacc, mybir
from concourse import bass_utils

N_CORES = 8
K = 8
H = 256
W = 256
HW = H * W          # 65536
P = 128
T = HW // P         # 512 pixel tiles
C = 32
V = 100000
VP = V // 4         # packed table rows (25000)
NPP = K * T         # 4096 lookups per partition

NI = 1024           # idxs per dma_gather
NQ = 4              # SWDGE queues (core pairs) used in parallel
GPC = 16            # gathers per DVE chunk (4 per queue)
CPC = GPC * NI // P  # blocks per DVE chunk = 128
CT = CPC // K       # pixel tiles per DVE chunk = 16
NCHUNK = T // CT    # 32 DVE chunks
NGATHER = NCHUNK * GPC  # 128

BF16 = ml_dtypes.bfloat16

LAST_EXEC_NS = None

_CACHE = {}


def _build_program():
    nc = bacc.Bacc("TRN2", target_bir_lowering=False, debug=False, num_swdge_queues=4)
    idx_d = nc.dram_tensor("idx16", [P, NPP * P // 16], mybir.dt.int16, kind="ExternalInput")
    wm_d = nc.dram_tensor("wm", [P, NCHUNK, 4 * CPC * 2], mybir.dt.bfloat16, kind="ExternalInput")
    feat_d = nc.dram_tensor("featP", [VP, 4 * C], mybir.dt.bfloat16, kind="ExternalInput")
    out_d = nc.dram_tensor("out", [P, T, C], mybir.dt.float32, kind="ExternalOutput")

    OP = mybir.AluOpType
    WMW = 4 * CPC * 2  # wm columns per chunk (1024)

    with (
        nc.Block() as block,
        ExitStack() as _es,
        nc.sbuf_tensor("idx_sb", [P, NPP * P // 16], mybir.dt.int16) as idx_sb,
        nc.sbuf_tensor("ga", [P, CPC, 4 * C], mybir.dt.bfloat16) as ga,
        nc.sbuf_tensor("gb", [P, CPC, 4 * C], mybir.dt.bfloat16) as gb,
        nc.sbuf_tensor("wma", [P, WMW], mybir.dt.bfloat16) as wma,
        nc.sbuf_tensor("wmb", [P, WMW], mybir.dt.bfloat16) as wmb,
        nc.sbuf_tensor("acc", [P, CPC, C], mybir.dt.bfloat16) as acc_sb,
        nc.sbuf_tensor("tmp", [P, CPC, C], mybir.dt.bfloat16) as tmp_sb,
        nc.sbuf_tensor("oa", [P, CT, C], mybir.dt.float32) as oa,
        nc.sbuf_tensor("ob", [P, CT, C], mybir.dt.float32) as ob,
    ):
        sem = lambda name: _es.enter_context(nc.semaphore(name))
        s_idx = sem("s_idx")
        s_gd = [[sem(f"s_gd{q}{p}") for q in range(NQ)] for p in "ab"]
        s_wm = [sem("s_wma"), sem("s_wmb")]
        s_dv = sem("s_dv")
        s_ow = [sem("s_owa"), sem("s_owb")]
        s_owa, s_owb = s_ow
        s_ch = sem("s_ch")

        gbufs = [ga, gb]
        wmbufs = [wma, wmb]
        obufs = [oa, ob]

        @block.gpsimd
        def _(g: bass.BassGpSimd):
            g.wait_ge(s_idx, 16)
            for c in range(NCHUNK):
                buf = gbufs[c % 2]
                if c >= 2:
                    g.wait_ge(s_dv, c - 1)
                for s in range(GPC):
                    gi = c * GPC + s
                    g.dma_gather(
                        buf[:, s * (NI // P):(s + 1) * (NI // P), :],
                        feat_d[:],
                        idx_sb[:, gi * (NI // 16):(gi + 1) * (NI // 16)],
                        NI,
                        NI,
                        4 * C,
                        single_packet=True,
                        queue_num=s % NQ,
                    ).then_inc(s_gd[c % 2][s % NQ], 16)

        @block.sync
        def _(sp: bass.BassEngine):
            sp.dma_start(idx_sb[:], idx_d[:]).then_inc(s_idx, 16)
            for c in range(NCHUNK):
                if c >= 2:
                    sp.wait_ge(s_dv, c - 1)
                sp.dma_start(wmbufs[c % 2][:], wm_d[:, c, :]).then_inc(
                    s_wm[c % 2], 16
                )
                if c >= 1:
                    sp.wait_ge(s_dv, c)
                    sp.dma_start(
                        out_d[:, (c - 1) * CT:c * CT, :], obufs[(c - 1) % 2][:]
                    ).then_inc(s_ow[(c - 1) % 2], 16)
            c = NCHUNK
            sp.wait_ge(s_dv, c)
            sp.dma_start(
                out_d[:, (c - 1) * CT:c * CT, :], obufs[(c - 1) % 2][:]
            ).then_inc(s_ow[(c - 1) % 2], 16)

        @block.vector
        def _(v: bass.BassEngine):
            # DVE has no same-engine RAW interlock (read-write bubble): chain
            # every op through s_ch (chunk-final ops inc s_dv instead, and the
            # next chunk's first op waits on s_dv).
            n_ch = 0
            for c in range(NCHUNK):
                for q in range(NQ):
                    v.wait_ge(s_gd[c % 2][q], 16 * (GPC // NQ) * (c // 2 + 1))
                v.wait_ge(s_wm[c % 2], 16 * (c // 2 + 1))
                if c >= 2:
                    v.wait_ge(s_ow[c % 2], 16 * (c // 2))
                if c >= 1:
                    v.wait_ge(s_dv, c)
                g = gbufs[c % 2]
                wmt = wmbufs[c % 2]
                o_sb = obufs[c % 2]
                for r in range(4):
                    g4 = g[:, :, r * C:(r + 1) * C].rearrange(
                        "p b (h d) -> p b h d", d=2
                    )
                    wb = (
                        wmt[:, r * (CPC * 2):(r + 1) * (CPC * 2)]
                        .rearrange("p (b d) -> p b d", d=2)[:, :, None, :]
                        .to_broadcast([P, CPC, C // 2, 2])
                    )
                    dst = (acc_sb if r == 0 else tmp_sb)[:].rearrange(
                        "p b (h d) -> p b h d", d=2
                    )
                    if n_ch > 0:
                        v.wait_ge(s_ch, n_ch)
                    v.tensor_tensor(out=dst, in0=g4, in1=wb, op=OP.mult).then_inc(
                        s_ch, 1
                    )
                    n_ch += 1
                    if r > 0:
                        v.wait_ge(s_ch, n_ch)
                        v.tensor_tensor(
                            out=acc_sb[:], in0=acc_sb[:], in1=tmp_sb[:], op=OP.add
                        ).then_inc(s_ch, 1)
                        n_ch += 1
                av = acc_sb[:].rearrange("p (t k) c -> p t k c", k=K)
                v.wait_ge(s_ch, n_ch)
                v.tensor_tensor(
                    out=av[:, :, 0:4, :], in0=av[:, :, 0:4, :], in1=av[:, :, 4:8, :],
                    op=OP.add,
                ).then_inc(s_ch, 1)
                n_ch += 1
                v.wait_ge(s_ch, n_ch)
                v.tensor_tensor(
                    out=av[:, :, 0:2, :], in0=av[:, :, 0:2, :], in1=av[:, :, 2:4, :],
                    op=OP.add,
                ).then_inc(s_ch, 1)
                n_ch += 1
                v.wait_ge(s_ch, n_ch)
                v.tensor_tensor(
                    out=o_sb[:], in0=av[:, :, 0, :], in1=av[:, :, 1, :], op=OP.add
                ).then_inc(s_dv, 1)
            v.wait_ge(s_owa, 16 * (NCHUNK // 2))
            v.wait_ge(s_owb, 16 * (NCHUNK // 2))

    nc.compile()
    return nc


def _get_program():
    if "nc" not in _CACHE:
        _CACHE["nc"] = _build_program()
    return _CACHE["nc"]


def _prep_core(frag, alph_w):
    """frag: (K,H,W) int32 clamped>=0; alph_w: (K,H,W) f32 weights.
    Returns idx16 [128, 32768] int16, wm [P, NCHUNK, 4*CPC*2] bf16."""
    fr = frag.reshape(K, T, P)
    wv = alph_w.reshape(K, T, P)
    # lookup order i = (t*8 + k)*128 + p
    j = (fr >> 2).astype(np.int16).transpose(1, 0, 2).reshape(-1)
    idx16 = np.ascontiguousarray(j.reshape(-1, 16).T)               # [16, 32768]
    idx16 = np.tile(idx16, (8, 1))                                  # [128, 32768]

    rs = (fr & 3).transpose(2, 1, 0).reshape(P, T * K)              # [p, b=t*8+k]
    wf = wv.transpose(2, 1, 0).reshape(P, T * K).astype(np.float32)
    wm = np.zeros((P, 4, T * K), dtype=np.float32)
    for r in range(4):
        wm[:, r, :] = np.where(rs == r, wf, 0.0)
    wm = wm.reshape(P, 4, NCHUNK, CPC).transpose(0, 2, 1, 3)        # [P, ch, 4, CPC]
    wm2 = np.repeat(wm[..., None], 2, axis=-1).astype(BF16)         # [P, ch, 4, CPC, 2]
    return idx16, np.ascontiguousarray(wm2.reshape(P, NCHUNK, 4 * CPC * 2))


def kernel(fragments, alphas, features):
    global LAST_EXEC_NS
    frag = np.asarray(fragments)
    alph = np.asarray(alphas, dtype=np.float32)
    feat = np.asarray(features, dtype=np.float32)
    n = frag.shape[0]
    assert frag.shape == (n, K, H, W) and alph.shape == (n, K, H, W)
    assert feat.shape == (C, V)

    a = np.where(frag >= 0, alph, 0.0).astype(np.float32)
    trans = np.cumprod(1.0 - a, axis=1)
    excl = np.concatenate([np.ones_like(trans[:, :1]), trans[:, :-1]], axis=1)
    wgt = a * excl

    fc = np.maximum(frag, 0).astype(np.int32)
    featP = np.ascontiguousarray(feat.T.reshape(VP, 4 * C)).astype(BF16)

    in_maps = []
    for i in range(n):
        idx16, wm2 = _prep_core(fc[i], wgt[i])
        in_maps.append({"idx16": idx16, "wm": wm2, "featP": featP})
    while len(in_maps) < N_CORES:
        in_maps.append(dict(in_maps[0]))

    nc = _get_program()
    trace = os.environ.get("BASS_KERNEL_TRACE", "0") == "1"
    res = bass_utils.run_bass_kernel_spmd(
        nc, in_maps, core_ids=list(range(N_CORES)), trace=trace
    )
    LAST_EXEC_NS = res.exec_time_ns

    out = np.empty((n, C, H, W), dtype=np.float32)
    for i in range(n):
        o = res.results[i]["out"]                 # [P, T, C]
        out[i] = o.transpose(2, 1, 0).reshape(C, H, W)
    return out
